# revision 4
# baseline (speedup 1.0000x reference)
"""DepthAttentionResidual Trainium2 kernel.

Computation (see reference):
    ms      = mean(history^2, axis=-1)                      # [S,B,T]
    logits  = dot(query*rms_weight, history) * rsqrt(ms+eps)
    w       = softmax(logits, axis=S)
    out     = sum_s w[s] * history[s]                        # [B,T,D]

Sharding: data-parallel over (B=4) x (T halves) = 8 cores. Each core gets
hist [S=16, Tc=1024, D=1024] (64 MiB) and produces out [1024, 1024].

Per-core layout: partition p = s*8 + t' (S=16 depths x 8 t-blocks), D on
the free axis. A supertile is 128 t; slice g holds t_local = t'*16 + g.

DMA: each SDMA engine sustains only ~12.5 GB/s per logical queue
(back-to-back packets, measured), so ONE HWDGE ring caps at ~200 GB/s.
The supertile is therefore loaded as TWO 4 MiB DMAs (32 KiB contiguous
per partition) on the two HWDGE rings (SP + ACT); engines round-robin
both packet streams, reaching the ~358 GB/s per-core HBM limit.

Compute per supertile (engines balanced against the ~24 us DMA budget):
  - sum(h^2) over D: ScalarE activation(Square, accum_out), last slice
    on VectorE
  - dot(q*w, h) over D: VectorE affine_mul_reduce
  - softmax over S: normalize-at-END: e = exp(dot * rstd) is used
    UN-normalized in the depth mix; Z[t] = sum_s e is produced by one
    extra accumulating PE matmul per slice (lhsT=w2, rhs=ones), and the
    PSUM->SBUF eviction scales by 1/Z[t] (per-partition scalar).
  - depth mix: per D-half, accumulating fp32r matmuls with
    block-expanded masked weights w2 built on GpSimd
    (w2[p, c] = e[p, g] iff c == t_local(p, g)).
First and last supertiles are split 64/64 to shorten the DMA ramp and
the serial tail. fp32r rounds operands to ~13 mantissa bits -> ~3e-4
relative output error.

Reads history exactly once (~68 MiB DMA per core): HBM-bound at
~200-220 us vs a ~190 us per-core HBM floor.
"""
import numpy as np

import concourse.bass as bass
import concourse.bacc as bacc
import concourse.tile as tile
from concourse import mybir
from concourse import bass_utils

N_CORES = 8
S = 16
B = 4
T = 2048
D = 1024
EPS = 1e-5

TC = T // 2          # t positions per core
TG = 8               # t-blocks per partition set (S * TG = 128 partitions)
GROUPS = 16          # stat slices per full supertile (one t per partition)
TS = TG * GROUPS     # t per supertile = 128
N_SUPER = TC // TS   # supertiles per core = 8
F32 = mybir.dt.float32
F32R = mybir.dt.float32r


def _build_program():
    nc = bacc.Bacc("TRN2", target_bir_lowering=False, debug=False,
                   enable_asserts=True, num_devices=N_CORES)

    hist = nc.dram_tensor("hist", [S, TC, D], F32R, kind="ExternalInput").ap()
    query = nc.dram_tensor("query", [D], F32, kind="ExternalInput").ap()
    rmsw = nc.dram_tensor("rms_weight", [D], F32, kind="ExternalInput").ap()
    maskf_d = nc.dram_tensor("maskF", [128, GROUPS, 128], F32,
                             kind="ExternalInput").ap()
    maskf8_d = nc.dram_tensor("maskF8", [128, GROUPS // 2, 128 // 2], F32,
                              kind="ExternalInput").ap()
    out = nc.dram_tensor("out", [TC, D], F32, kind="ExternalOutput").ap()

    with tile.TileContext(nc) as tc:
        with (
            tc.tile_pool(name="singles", bufs=1) as singles,
            tc.tile_pool(name="hsup", bufs=2) as hpool,
            tc.tile_pool(name="stats", bufs=2) as stats,
            tc.tile_pool(name="w2", bufs=3) as w2pool,
            tc.tile_pool(name="outp", bufs=2) as outpool,
            tc.tile_pool(name="ps_z", bufs=2, space="PSUM") as ps_z,
            tc.tile_pool(name="ps_mix", bufs=2, space="PSUM") as ps_mix,
        ):
            # ---- constants --------------------------------------------------
            qw = singles.tile([128, D], F32)
            wb = singles.tile([128, D], F32)
            maskF = singles.tile([128, GROUPS, 128], F32)
            maskF8 = singles.tile([128, GROUPS // 2, 128 // 2], F32)
            epst = singles.tile([128, 1], F32)
            ones1 = singles.tile([128, 2], F32)
            dummy_a = singles.tile([128, 1], F32)
            dummy_v = singles.tile([128, 1], F32)

            # small constants ride the ACT HWDGE ring ahead of the first
            # B-half input DMA; maskF (1 MiB, first needed by supertile 2)
            # is emitted after B0 so it never delays the input stream
            nc.scalar.dma_start(
                out=qw[:],
                in_=bass.AP(tensor=query.tensor, offset=0,
                            ap=[[0, 128], [1, D]]),
            )
            nc.scalar.dma_start(
                out=wb[:],
                in_=bass.AP(tensor=rmsw.tensor, offset=0,
                            ap=[[0, 128], [1, D]]),
            )
            nc.scalar.dma_start(out=maskF8[:], in_=maskf8_d)
            nc.vector.tensor_mul(qw[:], qw[:], wb[:])  # query * rms_weight
            nc.vector.memset(epst[:], EPS)
            nc.vector.memset(ones1[:], 1.0)

            # ---- main loop --------------------------------------------------
            # first and last supertiles are split into two 64-t halves
            schedule = [(0, GROUPS // 2), (TS // 2, GROUPS // 2)]
            schedule += [(k * TS, GROUPS) for k in range(1, N_SUPER - 1)]
            schedule += [((N_SUPER - 1) * TS, GROUPS // 2),
                         ((N_SUPER - 1) * TS + TS // 2, GROUPS // 2)]

            for k, (t0, groups) in enumerate(schedule):
                ts_k = TG * groups   # t positions in this entry
                jk = groups // 2     # slices per DMA (2 DMAs per entry)
                trows = TG * groups  # output rows

                # load [S, ts_k, D] as partitions (s, t') x free (g, d),
                # t_local = t' * groups + g; two DMAs (slices 0..jk-1 and
                # jk..groups-1), one per HWDGE ring, jk*4 KiB contiguous
                # per partition
                hsupA = hpool.tile([128, jk, D], F32R, tag="hsupA",
                                   name="hsupA")
                hsupB = hpool.tile([128, jk, D], F32R, tag="hsupB",
                                   name="hsupB")
                srcv = hist[:, t0:t0 + ts_k, :].rearrange(
                    "s (t gd j) d -> s t gd (j d)", t=TG, gd=2)
                nc.sync.dma_start(
                    out=hsupA.rearrange("p j d -> p (j d)"),
                    in_=srcv[:, :, 0, :])
                nc.scalar.dma_start(
                    out=hsupB.rearrange("p j d -> p (j d)"),
                    in_=srcv[:, :, 1, :])
                if k == 0:
                    nc.scalar.dma_start(out=maskF[:], in_=maskf_d)

                def hslice(g, jk=jk, hsupA=hsupA, hsupB=hsupB):
                    return (hsupA if g < jk else hsupB)[:, g % jk, :]

                ss = stats.tile([128, groups], F32, tag="ss")
                dot = stats.tile([128, groups], F32, tag="dot")
                for g in range(groups):
                    h_g = hslice(g).bitcast(F32)
                    if g < groups - 1:
                        nc.scalar.activation(
                            out=dummy_a.broadcast_to([128, D]),
                            in_=h_g,
                            func=mybir.ActivationFunctionType.Square,
                            accum_out=ss[:, g:g + 1],
                        )
                    else:
                        # last sumsq on VectorE keeps ScalarE under the
                        # DMA pace
                        nc.vector.affine_mul_reduce(
                            out=dummy_v.broadcast_to([128, D]),
                            accum_out=ss[:, g:g + 1],
                            in0=h_g, in1=h_g, scale=1.0, bias=0.0,
                        )
                    nc.vector.affine_mul_reduce(
                        out=dummy_v.broadcast_to([128, D]),
                        accum_out=dot[:, g:g + 1],
                        in0=h_g,
                        in1=qw[:],
                        scale=1.0,
                        bias=0.0,
                    )

                # rstd = 1/sqrt(ss/D + eps); logits = dot * rstd; e = exp
                sd = stats.tile([128, groups], F32, tag="sd")
                nc.scalar.activation(
                    out=sd[:], in_=ss[:],
                    func=mybir.ActivationFunctionType.Sqrt,
                    bias=epst[:], scale=1.0 / D,
                )
                rstd = stats.tile([128, groups], F32, tag="rstd")
                nc.vector.reciprocal(out=rstd[:], in_=sd[:])
                logit = stats.tile([128, groups], F32, tag="logit")
                nc.vector.tensor_mul(logit[:], dot[:], rstd[:])
                e = stats.tile([128, groups], F32, tag="e")
                nc.scalar.activation(
                    out=e[:], in_=logit[:],
                    func=mybir.ActivationFunctionType.Exp,
                )

                # depth mix with UN-normalized weights + Z accumulation:
                # m_ps[c][t, d] = sum_g sum_p w2[p, t] h[p, d]
                # z_ps[t, 0]    = sum_g sum_p w2[p, t]  (= sum_s e)
                m_ps = [ps_mix.tile([trows, 512], F32, tag="m",
                                    name=f"m{c}") for c in range(2)]
                z_ps = ps_z.tile([trows, 2], F32, tag="z")
                for g in range(groups):
                    w2 = w2pool.tile([128, trows], F32R, tag="w2")
                    nc.gpsimd.tensor_scalar(
                        out=w2[:],
                        in0=(maskF[:, g, :] if groups == GROUPS
                             else maskF8[:, g, :]),
                        scalar1=e[:, g:g + 1],
                        scalar2=None,
                        op0=mybir.AluOpType.mult,
                    )
                    for c in range(2):
                        nc.tensor.matmul(
                            out=m_ps[c][:],
                            lhsT=w2[:],
                            rhs=hslice(g)[:, c * 512:(c + 1) * 512],
                            start=(g == 0),
                            stop=(g == groups - 1),
                        )
                    nc.tensor.matmul(
                        out=z_ps[:],
                        lhsT=w2[:],
                        rhs=ones1.bitcast(F32R),
                        start=(g == 0),
                        stop=(g == groups - 1),
                    )

                # normalize during PSUM->SBUF eviction: ot = m_ps / Z
                rz = stats.tile([trows, 1], F32, tag="rz")
                nc.vector.reciprocal(out=rz[:], in_=z_ps[:, 0:1])
                ot = outpool.tile([trows, D], F32, tag="ot")
                nc.scalar.activation(
                    out=ot[:, 0:512], in_=m_ps[0][:],
                    func=mybir.ActivationFunctionType.Copy,
                    scale=rz[:, 0:1],
                )
                nc.vector.tensor_scalar(
                    out=ot[:, 512:1024], in0=m_ps[1][:],
                    scalar1=rz[:, 0:1], scalar2=None,
                    op0=mybir.AluOpType.mult,
                )
                nc.sync.dma_start(out=out[t0:t0 + ts_k, :], in_=ot[:])

    nc.compile()
    return nc


_NC = None


def _get_program():
    global _NC
    if _NC is None:
        _NC = _build_program()
    return _NC


def _make_masks():
    # partition p = s*TG + t'; slice g holds t_local = t'*groups + g
    p = np.arange(128)
    maskF = np.zeros((128, GROUPS, 128), np.float32)
    for g in range(GROUPS):
        maskF[p, g, (p % TG) * GROUPS + g] = 1.0
    maskF8 = np.zeros((128, GROUPS // 2, 64), np.float32)
    for g in range(GROUPS // 2):
        maskF8[p, g, (p % TG) * (GROUPS // 2) + g] = 1.0
    return maskF, maskF8


def _shard_inputs(nc, inputs):
    del nc
    maskF, maskF8 = _make_masks()
    history = np.asarray(inputs["history"], dtype=np.float32)
    query = np.asarray(inputs["query"], dtype=np.float32)
    rms_weight = np.asarray(inputs["rms_weight"], dtype=np.float32)
    in_maps = []
    for c in range(N_CORES):
        b, h = c // 2, c % 2
        shard = np.ascontiguousarray(history[:, b, h * TC:(h + 1) * TC, :])
        in_maps.append({
            "hist": shard,
            "query": query,
            "rms_weight": rms_weight,
            "maskF": maskF,
            "maskF8": maskF8,
        })
    return in_maps


def _expected_shard(expected, c):
    b, h = c // 2, c % 2
    return expected[b, h * TC:(h + 1) * TC, :]


def kernel(history, query, rms_weight):
    history = np.asarray(history, dtype=np.float32)
    query = np.asarray(query, dtype=np.float32)
    rms_weight = np.asarray(rms_weight, dtype=np.float32)
    assert history.shape == (S, B, T, D), history.shape

    nc = _get_program()
    in_maps = _shard_inputs(nc, {"history": history, "query": query,
                                 "rms_weight": rms_weight})
    res = bass_utils.run_bass_kernel_spmd(nc, in_maps, list(range(N_CORES)))

    out = np.empty((B, T, D), dtype=np.float32)
    for c in range(N_CORES):
        b, h = c // 2, c % 2
        out[b, h * TC:(h + 1) * TC, :] = res.results[c]["out"]
    return out


# revision 5
# speedup vs baseline: 1.0866x; 1.0866x over previous
"""DepthAttentionResidual Trainium2 kernel.

Computation (see reference):
    ms      = mean(history^2, axis=-1)                      # [S,B,T]
    logits  = dot(query*rms_weight, history) * rsqrt(ms+eps)
    w       = softmax(logits, axis=S)
    out     = sum_s w[s] * history[s]                        # [B,T,D]

Sharding: data-parallel over (B=4) x (T halves) = 8 cores. Each core gets
hist [S=16, Tc=1024, D=1024] (64 MiB) and produces out [1024, 1024].

Per-core layout: partition p = s*8 + t' (S=16 depths x 8 t-blocks), D on
the free axis. A supertile is 128 t; slice g holds t_local = t'*16 + g.

DMA: each SDMA engine sustains only ~12.5 GB/s per logical queue
(back-to-back packets, measured), so ONE HWDGE ring caps at ~200 GB/s.
The supertile is therefore loaded as TWO 4 MiB DMAs (32 KiB contiguous
per partition) on the two HWDGE rings (SP + ACT); the engines
round-robin both packet streams, approaching the ~358 GB/s per-core HBM
limit. The DMA triggers are software-pipelined one entry ahead so the
ACT-ring trigger for entry k+1 precedes entry k's ScalarE compute in
program order (otherwise the B-half stream serializes with compute).

Compute per supertile (engines balanced against the ~24 us DMA budget):
  - sum(h^2) over D: ScalarE activation(Square, accum_out), last slice
    on VectorE
  - dot(q*w, h) over D: VectorE affine_mul_reduce
  - softmax over S: normalize-at-END: e = exp(dot * rstd) is used
    UN-normalized in the depth mix; Z[t] = sum_s e is produced by one
    extra accumulating PE matmul per slice (lhsT=w2, rhs=ones), and the
    PSUM->SBUF eviction scales by 1/Z[t] (per-partition scalar).
  - depth mix: per D-half, accumulating fp32r matmuls with
    block-expanded masked weights (w2[p, c] = e[p, g] iff
    c == t_local(p, g)); all 16 w2 slices are built in ONE GpSimd
    tensor_tensor (maskF * e broadcast along the last axis).
First and last supertiles are split 64/64 to shorten the DMA ramp and
the serial tail. fp32r rounds operands to ~13 mantissa bits -> ~3e-4
relative output error.

Reads history exactly once (~68 MiB DMA per core): HBM-bound, ~190 us
per-core HBM floor.
"""
import numpy as np

import concourse.bass as bass
import concourse.bacc as bacc
import concourse.tile as tile
from concourse import mybir
from concourse import bass_utils

N_CORES = 8
S = 16
B = 4
T = 2048
D = 1024
EPS = 1e-5

TC = T // 2          # t positions per core
TG = 8               # t-blocks per partition set (S * TG = 128 partitions)
GROUPS = 16          # stat slices per full supertile (one t per partition)
TS = TG * GROUPS     # t per supertile = 128
N_SUPER = TC // TS   # supertiles per core = 8
F32 = mybir.dt.float32
F32R = mybir.dt.float32r


def _build_program():
    nc = bacc.Bacc("TRN2", target_bir_lowering=False, debug=False,
                   enable_asserts=True, num_devices=N_CORES)

    hist = nc.dram_tensor("hist", [S, TC, D], F32R, kind="ExternalInput").ap()
    query = nc.dram_tensor("query", [D], F32, kind="ExternalInput").ap()
    rmsw = nc.dram_tensor("rms_weight", [D], F32, kind="ExternalInput").ap()
    maskf_d = nc.dram_tensor("maskF", [128, GROUPS, 128], F32,
                             kind="ExternalInput").ap()
    maskf8_d = nc.dram_tensor("maskF8", [128, GROUPS // 2, 128 // 2], F32,
                              kind="ExternalInput").ap()
    out = nc.dram_tensor("out", [TC, D], F32, kind="ExternalOutput").ap()

    with tile.TileContext(nc) as tc:
        with (
            tc.tile_pool(name="singles", bufs=1) as singles,
            tc.tile_pool(name="hsup", bufs=2) as hpool,
            tc.tile_pool(name="stats", bufs=2) as stats,
            tc.tile_pool(name="w2", bufs=2) as w2pool,
            tc.tile_pool(name="outp", bufs=2) as outpool,
            tc.tile_pool(name="ps_z", bufs=2, space="PSUM") as ps_z,
            tc.tile_pool(name="ps_mix", bufs=2, space="PSUM") as ps_mix,
        ):
            # ---- constants --------------------------------------------------
            qw = singles.tile([128, D], F32)
            wb = singles.tile([128, D], F32)
            maskF = singles.tile([128, GROUPS, 128], F32)
            maskF8 = singles.tile([128, GROUPS // 2, 128 // 2], F32)
            epst = singles.tile([128, 1], F32)
            ones1 = singles.tile([128, 2], F32)
            dummy_a = singles.tile([128, 1], F32)
            dummy_v = singles.tile([128, 1], F32)

            # small constants ride the ACT HWDGE ring ahead of the first
            # B-half input DMA; maskF (1 MiB, first needed by supertile 2)
            # is emitted after B1 so it never delays the input stream
            nc.scalar.dma_start(
                out=qw[:],
                in_=bass.AP(tensor=query.tensor, offset=0,
                            ap=[[0, 128], [1, D]]),
            )
            nc.scalar.dma_start(
                out=wb[:],
                in_=bass.AP(tensor=rmsw.tensor, offset=0,
                            ap=[[0, 128], [1, D]]),
            )
            nc.scalar.dma_start(out=maskF8[:], in_=maskf8_d)
            nc.vector.tensor_mul(qw[:], qw[:], wb[:])  # query * rms_weight
            nc.vector.memset(epst[:], EPS)
            nc.vector.memset(ones1[:], 1.0)

            # ---- schedule ---------------------------------------------------
            # first and last supertiles are split into two 64-t halves
            schedule = [(0, GROUPS // 2), (TS // 2, GROUPS // 2)]
            schedule += [(k * TS, GROUPS) for k in range(1, N_SUPER - 1)]
            schedule += [((N_SUPER - 1) * TS, GROUPS // 2),
                         ((N_SUPER - 1) * TS + TS // 2, GROUPS // 2)]

            loads = {}

            def issue_load(k):
                t0, groups = schedule[k]
                jk = groups // 2
                hsupA = hpool.tile([128, jk, D], F32R, tag="hsupA",
                                   name="hsupA")
                hsupB = hpool.tile([128, jk, D], F32R, tag="hsupB",
                                   name="hsupB")
                srcv = hist[:, t0:t0 + TG * groups, :].rearrange(
                    "s (t gd j) d -> s t gd (j d)", t=TG, gd=2)
                nc.sync.dma_start(
                    out=hsupA.rearrange("p j d -> p (j d)"),
                    in_=srcv[:, :, 0, :])
                nc.scalar.dma_start(
                    out=hsupB.rearrange("p j d -> p (j d)"),
                    in_=srcv[:, :, 1, :])
                loads[k] = (hsupA, hsupB)

            issue_load(0)
            for k, (t0, groups) in enumerate(schedule):
                ts_k = TG * groups   # t positions in this entry
                jk = groups // 2
                trows = TG * groups  # output rows

                # prefetch: next entry's DMA triggers precede this entry's
                # compute in every engine's program order
                if k + 1 < len(schedule):
                    issue_load(k + 1)
                if k == 0:
                    nc.scalar.dma_start(out=maskF[:], in_=maskf_d)
                hsupA, hsupB = loads.pop(k)

                def hslice(g, jk=jk, hsupA=hsupA, hsupB=hsupB):
                    return (hsupA if g < jk else hsupB)[:, g % jk, :]

                ss = stats.tile([128, groups], F32, tag="ss")
                dot = stats.tile([128, groups], F32, tag="dot")
                for g in range(groups):
                    h_g = hslice(g).bitcast(F32)
                    if g < groups - 1:
                        nc.scalar.activation(
                            out=dummy_a.broadcast_to([128, D]),
                            in_=h_g,
                            func=mybir.ActivationFunctionType.Square,
                            accum_out=ss[:, g:g + 1],
                        )
                    else:
                        # last sumsq on VectorE keeps ScalarE under the
                        # DMA pace
                        nc.vector.affine_mul_reduce(
                            out=dummy_v.broadcast_to([128, D]),
                            accum_out=ss[:, g:g + 1],
                            in0=h_g, in1=h_g, scale=1.0, bias=0.0,
                        )
                    nc.vector.affine_mul_reduce(
                        out=dummy_v.broadcast_to([128, D]),
                        accum_out=dot[:, g:g + 1],
                        in0=h_g,
                        in1=qw[:],
                        scale=1.0,
                        bias=0.0,
                    )

                # rstd = 1/sqrt(ss/D + eps); logits = dot * rstd; e = exp
                sd = stats.tile([128, groups], F32, tag="sd")
                nc.scalar.activation(
                    out=sd[:], in_=ss[:],
                    func=mybir.ActivationFunctionType.Sqrt,
                    bias=epst[:], scale=1.0 / D,
                )
                rstd = stats.tile([128, groups], F32, tag="rstd")
                nc.vector.reciprocal(out=rstd[:], in_=sd[:])
                logit = stats.tile([128, groups], F32, tag="logit")
                nc.vector.tensor_mul(logit[:], dot[:], rstd[:])
                e = stats.tile([128, groups], F32, tag="e")
                nc.scalar.activation(
                    out=e[:], in_=logit[:],
                    func=mybir.ActivationFunctionType.Exp,
                )

                # all w2 slices in one GpSimd op:
                # w2all[p, g, c] = maskF[p, g, c] * e[p, g]
                w2all = w2pool.tile([128, groups, trows], F32R, tag="w2")
                nc.gpsimd.tensor_tensor(
                    out=w2all[:],
                    in0=(maskF[:] if groups == GROUPS else maskF8[:]),
                    in1=e[:].unsqueeze(2).broadcast_to([128, groups, trows]),
                    op=mybir.AluOpType.mult,
                )

                # depth mix with UN-normalized weights + Z accumulation:
                # m_ps[c][t, d] = sum_g sum_p w2[p, t] h[p, d]
                # z_ps[t, 0]    = sum_g sum_p w2[p, t]  (= sum_s e)
                m_ps = [ps_mix.tile([trows, 512], F32, tag="m",
                                    name=f"m{c}") for c in range(2)]
                z_ps = ps_z.tile([trows, 2], F32, tag="z")
                for g in range(groups):
                    w2g = w2all[:, g, :]
                    for c in range(2):
                        nc.tensor.matmul(
                            out=m_ps[c][:],
                            lhsT=w2g,
                            rhs=hslice(g)[:, c * 512:(c + 1) * 512],
                            start=(g == 0),
                            stop=(g == groups - 1),
                        )
                    nc.tensor.matmul(
                        out=z_ps[:],
                        lhsT=w2g,
                        rhs=ones1.bitcast(F32R),
                        start=(g == 0),
                        stop=(g == groups - 1),
                    )

                # normalize during PSUM->SBUF eviction: ot = m_ps / Z
                rz = stats.tile([trows, 1], F32, tag="rz")
                nc.vector.reciprocal(out=rz[:], in_=z_ps[:, 0:1])
                ot = outpool.tile([trows, D], F32, tag="ot")
                nc.scalar.activation(
                    out=ot[:, 0:512], in_=m_ps[0][:],
                    func=mybir.ActivationFunctionType.Copy,
                    scale=rz[:, 0:1],
                )
                nc.vector.tensor_scalar(
                    out=ot[:, 512:1024], in0=m_ps[1][:],
                    scalar1=rz[:, 0:1], scalar2=None,
                    op0=mybir.AluOpType.mult,
                )
                nc.sync.dma_start(out=out[t0:t0 + ts_k, :], in_=ot[:])

    nc.compile()
    return nc


_NC = None


def _get_program():
    global _NC
    if _NC is None:
        _NC = _build_program()
    return _NC


def _make_masks():
    # partition p = s*TG + t'; slice g holds t_local = t'*groups + g
    p = np.arange(128)
    maskF = np.zeros((128, GROUPS, 128), np.float32)
    for g in range(GROUPS):
        maskF[p, g, (p % TG) * GROUPS + g] = 1.0
    maskF8 = np.zeros((128, GROUPS // 2, 64), np.float32)
    for g in range(GROUPS // 2):
        maskF8[p, g, (p % TG) * (GROUPS // 2) + g] = 1.0
    return maskF, maskF8


def _shard_inputs(nc, inputs):
    del nc
    maskF, maskF8 = _make_masks()
    history = np.asarray(inputs["history"], dtype=np.float32)
    query = np.asarray(inputs["query"], dtype=np.float32)
    rms_weight = np.asarray(inputs["rms_weight"], dtype=np.float32)
    in_maps = []
    for c in range(N_CORES):
        b, h = c // 2, c % 2
        shard = np.ascontiguousarray(history[:, b, h * TC:(h + 1) * TC, :])
        in_maps.append({
            "hist": shard,
            "query": query,
            "rms_weight": rms_weight,
            "maskF": maskF,
            "maskF8": maskF8,
        })
    return in_maps


def _expected_shard(expected, c):
    b, h = c // 2, c % 2
    return expected[b, h * TC:(h + 1) * TC, :]


def kernel(history, query, rms_weight):
    history = np.asarray(history, dtype=np.float32)
    query = np.asarray(query, dtype=np.float32)
    rms_weight = np.asarray(rms_weight, dtype=np.float32)
    assert history.shape == (S, B, T, D), history.shape

    nc = _get_program()
    in_maps = _shard_inputs(nc, {"history": history, "query": query,
                                 "rms_weight": rms_weight})
    res = bass_utils.run_bass_kernel_spmd(nc, in_maps, list(range(N_CORES)))

    out = np.empty((B, T, D), dtype=np.float32)
    for c in range(N_CORES):
        b, h = c // 2, c % 2
        out[b, h * TC:(h + 1) * TC, :] = res.results[c]["out"]
    return out


# revision 6
# speedup vs baseline: 1.5569x; 1.4328x over previous
"""DepthAttentionResidual Trainium2 kernel (fp16 t-layout).

Computation (see reference):
    ms      = mean(history^2, axis=-1)                      # [S,B,T]
    logits  = dot(query*rms_weight, history) * rsqrt(ms+eps)
    w       = softmax(logits, axis=S)
    out     = sum_s w[s] * history[s]                        # [B,T,D]

Sharding: data-parallel over (B=4) x (T halves) = 8 cores. Each core gets
hist [S=16, Tc=1024, D=1024] (64 MiB f32) and produces out [1024, 1024].

Bandwidth model (measured on this part): every SDMA engine moves only
~13 GB/s of SBUF-side bytes per stream regardless of queue count or
packet size (strict 2:1 port slotting), so a plain f32 load caps at
~210 GB/s/core -> 315 us. The SWDGE (GpSimd) DMA path can CAST
f32->fp16 in the datapath, halving SBUF-side bytes: the same stream
then carries ~376 GB/s of HBM-side bytes. All history loads are SWDGE
cast-DMAs; fp16 keeps ~5e-4 output accuracy (gate is 2e-2).

Layout: partition p = t (128 t per supertile), free = (s, d). A
supertile is [16 s][128 t][1024 d], loaded as two cast-DMAs (s 0-7,
s 8-15), triggers software-pipelined one supertile ahead on GpSimd.

Per supertile (engines balanced against the ~21 us DMA budget):
  - sumsq over D: ScalarE Square+accum (s0-11); GpSimd squares into
    fp16 temps for s12-15, reduced by one grouped VectorE tensor_reduce
  - dot(qw, h): VectorE affine_mul_reduce (s0-13); GpSimd multiplies +
    grouped reduce for s14-15
  - softmax over S is a ROW op here: rstd via ACT Sqrt + DVE
    reciprocal; logits = dot*rstd on GpSimd; e = ACT Exp -> fp16.
    Weights stay UN-normalized: Z = row-sum(e) (one small DVE reduce),
    and the PSUM->SBUF eviction scales by 1/Z (per-partition scalar).
  - depth mix: per D-half, 16 accumulating fp16 matmuls with diagonal
    masked weights wD_s = diag(e[:, s]) (all 16 built in one GpSimd
    tensor_tensor from a diagonal-mask constant).
The last supertile runs softmax/w2 per s-half to shorten the tail.

Reads history exactly once; ~64 MiB HBM + 2 MiB constants in, 4 MiB
out per core; ~180 us DMA floor, engines at ~21-23 us per supertile.
"""
import numpy as np

import concourse.bass as bass
import concourse.bacc as bacc
import concourse.tile as tile
from concourse import mybir
from concourse import bass_utils

N_CORES = 8
S = 16
B = 4
T = 2048
D = 1024
EPS = 1e-5

TC = T // 2          # t positions per core
TS = 128             # t per supertile (= partition count)
N_SUPER = TC // TS   # supertiles per core = 8
SH = S // 2          # s per DMA half
F32 = mybir.dt.float32
F16 = mybir.dt.float16

# stats engine split (full supertile): ACT squares s0..NSQ_ACT-1,
# GpSimd squares the rest; DVE dots s0..NDOT_DVE-1, GpSimd the rest.
NSQ_ACT = 12
NDOT_DVE = 14


def _build_program():
    nc = bacc.Bacc("TRN2", target_bir_lowering=False, debug=False,
                   enable_asserts=True, num_devices=N_CORES)

    hist = nc.dram_tensor("hist", [S, TC, D], F32, kind="ExternalInput").ap()
    query = nc.dram_tensor("query", [D], F32, kind="ExternalInput").ap()
    rmsw = nc.dram_tensor("rms_weight", [D], F32, kind="ExternalInput").ap()
    maskd_d = nc.dram_tensor("maskD", [128, S, 128], F16,
                             kind="ExternalInput").ap()
    out = nc.dram_tensor("out", [TC, D], F32, kind="ExternalOutput").ap()

    with tile.TileContext(nc) as tc:
        with (
            tc.tile_pool(name="singles", bufs=1) as singles,
            tc.tile_pool(name="hsup", bufs=3) as hpool,
            tc.tile_pool(name="stats", bufs=2) as stats,
            tc.tile_pool(name="gtmp", bufs=2) as gpool,
            tc.tile_pool(name="w2", bufs=2) as w2pool,
            tc.tile_pool(name="outp", bufs=2) as outpool,
            tc.tile_pool(name="ps_mix", bufs=2, space="PSUM") as ps_mix,
        ):
            # ---- constants --------------------------------------------------
            qw = singles.tile([128, D], F32)
            wb = singles.tile([128, D], F32)
            qwh = singles.tile([128, D], F16)
            maskD = singles.tile([128, S, 128], F16)
            epst = singles.tile([128, 1], F32)
            dummy_a = singles.tile([128, 1], F32)
            dummy_v = singles.tile([128, 1], F32)

            nc.scalar.dma_start(
                out=qw[:],
                in_=bass.AP(tensor=query.tensor, offset=0,
                            ap=[[0, 128], [1, D]]),
            )
            nc.scalar.dma_start(
                out=wb[:],
                in_=bass.AP(tensor=rmsw.tensor, offset=0,
                            ap=[[0, 128], [1, D]]),
            )
            nc.scalar.dma_start(out=maskD[:], in_=maskd_d)
            nc.vector.tensor_mul(qw[:], qw[:], wb[:])   # query * rms_weight
            nc.vector.tensor_copy(out=qwh[:], in_=qw[:])  # -> fp16
            nc.vector.memset(epst[:], EPS)

            loads = {}

            def issue_load(k):
                t0 = k * TS
                hA = hpool.tile([128, SH, D], F16, tag="hA", name="hA")
                hB = hpool.tile([128, SH, D], F16, tag="hB", name="hB")
                nc.gpsimd.dma_start(
                    out=hA[:],
                    in_=hist[0:SH, t0:t0 + TS, :].rearrange("s t d -> t s d"))
                nc.gpsimd.dma_start(
                    out=hB[:],
                    in_=hist[SH:S, t0:t0 + TS, :].rearrange("s t d -> t s d"))
                loads[k] = (hA, hB)

            issue_load(0)
            for k in range(N_SUPER):
                t0 = k * TS
                if k + 1 < N_SUPER:
                    issue_load(k + 1)
                hA, hB = loads.pop(k)
                last = (k == N_SUPER - 1)

                def hslice(s, hA=hA, hB=hB):
                    return (hA if s < SH else hB)[:, s % SH, :]

                # ---- stats: ss[t, s] = sum_d h^2, dot[t, s] = sum_d h*qw
                ss = stats.tile([128, S], F32, tag="ss")
                dot = stats.tile([128, S], F32, tag="dot")
                gsq = gpool.tile([128, S - NSQ_ACT, D], F16, tag="gsq")
                gdot = gpool.tile([128, S - NDOT_DVE, D], F16, tag="gdot")
                for s in range(S):
                    h_s = hslice(s)
                    if s < NSQ_ACT:
                        nc.scalar.activation(
                            out=dummy_a.broadcast_to([128, D]),
                            in_=h_s,
                            func=mybir.ActivationFunctionType.Square,
                            accum_out=ss[:, s:s + 1],
                        )
                    else:
                        nc.gpsimd.tensor_tensor(
                            out=gsq[:, s - NSQ_ACT, :], in0=h_s, in1=h_s,
                            op=mybir.AluOpType.mult)
                    if s < NDOT_DVE:
                        nc.vector.affine_mul_reduce(
                            out=dummy_v.broadcast_to([128, D]),
                            accum_out=dot[:, s:s + 1],
                            in0=h_s, in1=qwh[:], scale=1.0, bias=0.0,
                        )
                    else:
                        nc.gpsimd.tensor_tensor(
                            out=gdot[:, s - NDOT_DVE, :], in0=h_s, in1=qwh[:],
                            op=mybir.AluOpType.mult)
                nc.vector.tensor_reduce(
                    out=ss[:, NSQ_ACT:S], in_=gsq[:],
                    axis=mybir.AxisListType.X, op=mybir.AluOpType.add)
                nc.vector.tensor_reduce(
                    out=dot[:, NDOT_DVE:S], in_=gdot[:],
                    axis=mybir.AxisListType.X, op=mybir.AluOpType.add)

                # ---- softmax numerator: e = exp(dot * rsqrt(ss/D + eps))
                sd = stats.tile([128, S], F32, tag="sd")
                rstd = stats.tile([128, S], F32, tag="rstd")
                logit = stats.tile([128, S], F32, tag="logit")
                e = stats.tile([128, S], F16, tag="e")
                w2all = w2pool.tile([128, S, 128], F16, tag="w2")

                def softmax_cols(c0, c1):
                    nc.scalar.activation(
                        out=sd[:, c0:c1], in_=ss[:, c0:c1],
                        func=mybir.ActivationFunctionType.Sqrt,
                        bias=epst[:], scale=1.0 / D,
                    )
                    nc.vector.reciprocal(out=rstd[:, c0:c1], in_=sd[:, c0:c1])
                    nc.gpsimd.tensor_mul(logit[:, c0:c1], dot[:, c0:c1],
                                         rstd[:, c0:c1])
                    nc.scalar.activation(
                        out=e[:, c0:c1], in_=logit[:, c0:c1],
                        func=mybir.ActivationFunctionType.Exp,
                    )
                    nc.gpsimd.tensor_tensor(
                        out=w2all[:, c0:c1, :],
                        in0=maskD[:, c0:c1, :],
                        in1=e[:, c0:c1].unsqueeze(2)
                            .broadcast_to([128, c1 - c0, 128]),
                        op=mybir.AluOpType.mult,
                    )

                if last:
                    # split per s-half: shortens the serial tail after the
                    # final DMA (mix of s0-7 starts before s8-15 stats end)
                    softmax_cols(0, SH)
                    softmax_cols(SH, S)
                else:
                    softmax_cols(0, S)

                # ---- depth mix: m_ps[c][t, d] += e[t, s] * h_s[t, d]
                m_ps = [ps_mix.tile([128, 512], F32, tag="m",
                                    name=f"m{c}") for c in range(2)]
                for s in range(S):
                    for c in range(2):
                        nc.tensor.matmul(
                            out=m_ps[c][:],
                            lhsT=w2all[:, s, :],
                            rhs=hslice(s)[:, c * 512:(c + 1) * 512],
                            start=(s == 0),
                            stop=(s == S - 1),
                        )

                # ---- normalize during eviction: ot = m_ps / Z, Z = sum_s e
                zt = stats.tile([128, 1], F32, tag="zt")
                nc.vector.tensor_reduce(
                    out=zt[:], in_=e[:],
                    axis=mybir.AxisListType.X, op=mybir.AluOpType.add)
                rz = stats.tile([128, 1], F32, tag="rz")
                nc.vector.reciprocal(out=rz[:], in_=zt[:])
                ot = outpool.tile([128, D], F32, tag="ot")
                nc.scalar.activation(
                    out=ot[:, 0:512], in_=m_ps[0][:],
                    func=mybir.ActivationFunctionType.Copy,
                    scale=rz[:, 0:1],
                )
                nc.scalar.activation(
                    out=ot[:, 512:1024], in_=m_ps[1][:],
                    func=mybir.ActivationFunctionType.Copy,
                    scale=rz[:, 0:1],
                )
                nc.sync.dma_start(out=out[t0:t0 + TS, :], in_=ot[:])

    nc.compile()
    return nc


_NC = None


def _get_program():
    global _NC
    if _NC is None:
        _NC = _build_program()
    return _NC


def _make_masks():
    p = np.arange(128)
    maskD = np.zeros((128, S, 128), np.float16)
    maskD[p, :, p] = 1.0
    return maskD


def _shard_inputs(nc, inputs):
    del nc
    maskD = _make_masks()
    history = np.asarray(inputs["history"], dtype=np.float32)
    query = np.asarray(inputs["query"], dtype=np.float32)
    rms_weight = np.asarray(inputs["rms_weight"], dtype=np.float32)
    in_maps = []
    for c in range(N_CORES):
        b, h = c // 2, c % 2
        shard = np.ascontiguousarray(history[:, b, h * TC:(h + 1) * TC, :])
        in_maps.append({
            "hist": shard,
            "query": query,
            "rms_weight": rms_weight,
            "maskD": maskD,
        })
    return in_maps


def _expected_shard(expected, c):
    b, h = c // 2, c % 2
    return expected[b, h * TC:(h + 1) * TC, :]


def kernel(history, query, rms_weight):
    history = np.asarray(history, dtype=np.float32)
    query = np.asarray(query, dtype=np.float32)
    rms_weight = np.asarray(rms_weight, dtype=np.float32)
    assert history.shape == (S, B, T, D), history.shape

    nc = _get_program()
    in_maps = _shard_inputs(nc, {"history": history, "query": query,
                                 "rms_weight": rms_weight})
    res = bass_utils.run_bass_kernel_spmd(nc, in_maps, list(range(N_CORES)))

    out = np.empty((B, T, D), dtype=np.float32)
    for c in range(N_CORES):
        b, h = c // 2, c % 2
        out[b, h * TC:(h + 1) * TC, :] = res.results[c]["out"]
    return out


# revision 7
# speedup vs baseline: 2.2192x; 1.4254x over previous
"""DepthAttentionResidual Trainium2 kernel (fp16 t-layout).

Computation (see reference):
    ms      = mean(history^2, axis=-1)                      # [S,B,T]
    logits  = dot(query*rms_weight, history) * rsqrt(ms+eps)
    w       = softmax(logits, axis=S)
    out     = sum_s w[s] * history[s]                        # [B,T,D]

Sharding: data-parallel over (B=4) x (T halves) = 8 cores. Each core gets
hist [S=16, Tc=1024, D=1024] (64 MiB f32) and produces out [1024, 1024].

Bandwidth model (measured on this part): every SDMA engine moves only
~13 GB/s of SBUF-side bytes per stream regardless of queue count or
packet size (strict 2:1 port slotting), so a plain f32 load caps at
~210 GB/s/core -> 315 us. The SWDGE (GpSimd) DMA path can CAST
f32->fp16 in the datapath, halving SBUF-side bytes: the same stream
then carries ~376 GB/s of HBM-side bytes. All history loads are SWDGE
cast-DMAs; fp16 keeps ~5e-4 output accuracy (gate is 2e-2).

Layout: partition p = t (128 t per supertile), free = (s, d). A
supertile is [16 s][128 t][1024 d], loaded as two cast-DMAs (s 0-7,
s 8-15), triggers software-pipelined one supertile ahead on GpSimd.

Per supertile (engines balanced against the ~21 us DMA budget):
  - sumsq over D: ScalarE Square+accum (s0-11); GpSimd squares into
    fp16 temps for s12-15, reduced by one grouped VectorE tensor_reduce
  - dot(qw, h): VectorE affine_mul_reduce (s0-13); GpSimd multiplies +
    grouped reduce for s14-15
  - softmax over S is a ROW op here: rstd via ACT Sqrt + DVE
    reciprocal; logits = dot*rstd on GpSimd; e = ACT Exp -> fp16.
    Weights stay UN-normalized: Z = row-sum(e) (one small DVE reduce),
    and the PSUM->SBUF eviction scales by 1/Z (per-partition scalar).
  - depth mix: per D-half, 16 accumulating fp16 matmuls with diagonal
    masked weights wD_s = diag(e[:, s]) (all 16 built in one GpSimd
    tensor_tensor from a diagonal-mask constant).
The last supertile runs softmax/w2 per s-half to shorten the tail.

Reads history exactly once; ~64 MiB HBM + 2 MiB constants in, 4 MiB
out per core; ~180 us DMA floor, engines at ~21-23 us per supertile.
"""
import numpy as np

import concourse.bass as bass
import concourse.bacc as bacc
import concourse.tile as tile
from concourse import mybir
from concourse import bass_utils

N_CORES = 8
S = 16
B = 4
T = 2048
D = 1024
EPS = 1e-5

TC = T // 2          # t positions per core
TS = 128             # t per supertile (= partition count)
N_SUPER = TC // TS   # supertiles per core = 8
SH = S // 2          # s per DMA half
F32 = mybir.dt.float32
F16 = mybir.dt.float16

# stats engine split (full supertile): ACT squares s0..NSQ_ACT-1, DVE
# (affine_mul_reduce) the rest; DVE does all dots. GpSimd only runs the
# DMA triggers, the logit multiply and the w2 build -- anything bigger
# there both stalls the software-pipelined SWDGE triggers and fights
# VectorE for their shared SBUF port.
NSQ_ACT = 15


def _build_program():
    nc = bacc.Bacc("TRN2", target_bir_lowering=False, debug=False,
                   enable_asserts=True, num_devices=N_CORES)

    hist = nc.dram_tensor("hist", [S, TC, D], F32, kind="ExternalInput").ap()
    query = nc.dram_tensor("query", [D], F32, kind="ExternalInput").ap()
    rmsw = nc.dram_tensor("rms_weight", [D], F32, kind="ExternalInput").ap()
    maskd_d = nc.dram_tensor("maskD", [128, S, 128], F16,
                             kind="ExternalInput").ap()
    out = nc.dram_tensor("out", [TC, D], F32, kind="ExternalOutput").ap()

    with tile.TileContext(nc) as tc:
        with (
            tc.tile_pool(name="singles", bufs=1) as singles,
            tc.tile_pool(name="hsup", bufs=4) as hpool,
            tc.tile_pool(name="stats", bufs=2) as stats,
            tc.tile_pool(name="w2", bufs=2) as w2pool,
            tc.tile_pool(name="outp", bufs=2) as outpool,
            tc.tile_pool(name="ps_mix", bufs=2, space="PSUM") as ps_mix,
        ):
            # ---- constants --------------------------------------------------
            qw = singles.tile([128, D], F32)
            wb = singles.tile([128, D], F32)
            qwh = singles.tile([128, D], F16)
            maskD = singles.tile([128, S, 128], F16)
            epst = singles.tile([128, 1], F32)
            dummy_a = singles.tile([128, 1], F32)
            dummy_v = singles.tile([128, 1], F32)

            nc.scalar.dma_start(
                out=qw[:],
                in_=bass.AP(tensor=query.tensor, offset=0,
                            ap=[[0, 128], [1, D]]),
            )
            nc.scalar.dma_start(
                out=wb[:],
                in_=bass.AP(tensor=rmsw.tensor, offset=0,
                            ap=[[0, 128], [1, D]]),
            )
            nc.scalar.dma_start(out=maskD[:], in_=maskd_d)
            nc.vector.tensor_mul(qw[:], qw[:], wb[:])   # query * rms_weight
            nc.vector.tensor_copy(out=qwh[:], in_=qw[:])  # -> fp16
            nc.vector.memset(epst[:], EPS)

            loads = {}

            def issue_load(k):
                t0 = k * TS
                hA = hpool.tile([128, SH, D], F16, tag="hA", name="hA")
                hB = hpool.tile([128, SH, D], F16, tag="hB", name="hB")
                nc.gpsimd.dma_start(
                    out=hA[:],
                    in_=hist[0:SH, t0:t0 + TS, :].rearrange("s t d -> t s d"))
                nc.gpsimd.dma_start(
                    out=hB[:],
                    in_=hist[SH:S, t0:t0 + TS, :].rearrange("s t d -> t s d"))
                loads[k] = (hA, hB)

            issue_load(0)
            issue_load(1)
            for k in range(N_SUPER):
                t0 = k * TS
                if k + 2 < N_SUPER:
                    issue_load(k + 2)
                hA, hB = loads.pop(k)
                last = (k == N_SUPER - 1)

                def hslice(s, hA=hA, hB=hB):
                    return (hA if s < SH else hB)[:, s % SH, :]

                # ---- stats: ss[t, s] = sum_d h^2, dot[t, s] = sum_d h*qw
                ss = stats.tile([128, S], F32, tag="ss")
                dot = stats.tile([128, S], F32, tag="dot")
                for s in range(S):
                    h_s = hslice(s)
                    if s < NSQ_ACT:
                        nc.scalar.activation(
                            out=dummy_a.broadcast_to([128, D]),
                            in_=h_s,
                            func=mybir.ActivationFunctionType.Square,
                            accum_out=ss[:, s:s + 1],
                        )
                    else:
                        nc.vector.affine_mul_reduce(
                            out=dummy_v.broadcast_to([128, D]),
                            accum_out=ss[:, s:s + 1],
                            in0=h_s, in1=h_s, scale=1.0, bias=0.0,
                        )
                    nc.vector.affine_mul_reduce(
                        out=dummy_v.broadcast_to([128, D]),
                        accum_out=dot[:, s:s + 1],
                        in0=h_s, in1=qwh[:], scale=1.0, bias=0.0,
                    )

                # ---- softmax numerator: e = exp(dot * rsqrt(ss/D + eps))
                sd = stats.tile([128, S], F32, tag="sd")
                rstd = stats.tile([128, S], F32, tag="rstd")
                logit = stats.tile([128, S], F32, tag="logit")
                e = stats.tile([128, S], F16, tag="e")
                w2all = w2pool.tile([128, S, 128], F16, tag="w2")

                def softmax_cols(c0, c1):
                    nc.scalar.activation(
                        out=sd[:, c0:c1], in_=ss[:, c0:c1],
                        func=mybir.ActivationFunctionType.Sqrt,
                        bias=epst[:], scale=1.0 / D,
                    )
                    nc.vector.reciprocal(out=rstd[:, c0:c1], in_=sd[:, c0:c1])
                    nc.gpsimd.tensor_mul(logit[:, c0:c1], dot[:, c0:c1],
                                         rstd[:, c0:c1])
                    nc.scalar.activation(
                        out=e[:, c0:c1], in_=logit[:, c0:c1],
                        func=mybir.ActivationFunctionType.Exp,
                    )
                    nc.gpsimd.tensor_tensor(
                        out=w2all[:, c0:c1, :],
                        in0=maskD[:, c0:c1, :],
                        in1=e[:, c0:c1].unsqueeze(2)
                            .broadcast_to([128, c1 - c0, 128]),
                        op=mybir.AluOpType.mult,
                    )

                if last:
                    # split per s-half: shortens the serial tail after the
                    # final DMA (mix of s0-7 starts before s8-15 stats end)
                    softmax_cols(0, SH)
                    softmax_cols(SH, S)
                else:
                    softmax_cols(0, S)

                # ---- depth mix: m_ps[c][t, d] += e[t, s] * h_s[t, d]
                m_ps = [ps_mix.tile([128, 512], F32, tag="m",
                                    name=f"m{c}") for c in range(2)]
                for s in range(S):
                    for c in range(2):
                        nc.tensor.matmul(
                            out=m_ps[c][:],
                            lhsT=w2all[:, s, :],
                            rhs=hslice(s)[:, c * 512:(c + 1) * 512],
                            start=(s == 0),
                            stop=(s == S - 1),
                        )

                # ---- normalize during eviction: ot = m_ps / Z, Z = sum_s e
                zt = stats.tile([128, 1], F32, tag="zt")
                nc.vector.tensor_reduce(
                    out=zt[:], in_=e[:],
                    axis=mybir.AxisListType.X, op=mybir.AluOpType.add)
                rz = stats.tile([128, 1], F32, tag="rz")
                nc.vector.reciprocal(out=rz[:], in_=zt[:])
                ot = outpool.tile([128, D], F32, tag="ot")
                nc.scalar.activation(
                    out=ot[:, 0:512], in_=m_ps[0][:],
                    func=mybir.ActivationFunctionType.Copy,
                    scale=rz[:, 0:1],
                )
                nc.scalar.activation(
                    out=ot[:, 512:1024], in_=m_ps[1][:],
                    func=mybir.ActivationFunctionType.Copy,
                    scale=rz[:, 0:1],
                )
                nc.sync.dma_start(out=out[t0:t0 + TS, :], in_=ot[:])

    nc.compile()
    return nc


_NC = None


def _get_program():
    global _NC
    if _NC is None:
        _NC = _build_program()
    return _NC


def _make_masks():
    p = np.arange(128)
    maskD = np.zeros((128, S, 128), np.float16)
    maskD[p, :, p] = 1.0
    return maskD


def _shard_inputs(nc, inputs):
    del nc
    maskD = _make_masks()
    history = np.asarray(inputs["history"], dtype=np.float32)
    query = np.asarray(inputs["query"], dtype=np.float32)
    rms_weight = np.asarray(inputs["rms_weight"], dtype=np.float32)
    in_maps = []
    for c in range(N_CORES):
        b, h = c // 2, c % 2
        shard = np.ascontiguousarray(history[:, b, h * TC:(h + 1) * TC, :])
        in_maps.append({
            "hist": shard,
            "query": query,
            "rms_weight": rms_weight,
            "maskD": maskD,
        })
    return in_maps


def _expected_shard(expected, c):
    b, h = c // 2, c % 2
    return expected[b, h * TC:(h + 1) * TC, :]


def kernel(history, query, rms_weight):
    history = np.asarray(history, dtype=np.float32)
    query = np.asarray(query, dtype=np.float32)
    rms_weight = np.asarray(rms_weight, dtype=np.float32)
    assert history.shape == (S, B, T, D), history.shape

    nc = _get_program()
    in_maps = _shard_inputs(nc, {"history": history, "query": query,
                                 "rms_weight": rms_weight})
    res = bass_utils.run_bass_kernel_spmd(nc, in_maps, list(range(N_CORES)))

    out = np.empty((B, T, D), dtype=np.float32)
    for c in range(N_CORES):
        b, h = c // 2, c % 2
        out[b, h * TC:(h + 1) * TC, :] = res.results[c]["out"]
    return out
